# revision 6
# baseline (speedup 1.0000x reference)
"""Trainium2 Bass kernel for nn_CosineProxy.

Reference computation (per task b):
    feats[n]  = blockmean_pool(x[b,n])            # (640,10,10) -> 800 dims
    proxy     = sum_n feats[n]                     # pooling is linear
    sim[n]    = <feats[n], proxy> / max(||feats[n]||*||proxy||, eps)
    out[b]    = sum_n sim[n] * x[b,n]

sim is scale-invariant, so block-SUM pooling is used instead of block-mean.
Sharding: pure data parallelism over B=256 tasks -> 32 tasks per core x 8 cores.

Per-core layout: x[b,n] (640*100 contiguous floats) lives in SBUF as
(128 partitions, 500 free) where partition p holds channels [5p,5p+5).

v2 pipeline per group of 4 tasks, tuned for the DMA roofline (~137us/core):
  1. DVE: 2x2 spatial pooling as two strided tensor_tensor adds per task,
     emitting fp16 (s2 runs at the 16-bit 2x DVE rate).
  2. PE: fp16 "packing" matmuls (lhsT = block-indicator) channel-pool the
     4 tasks into two PSUM banks (8 matmuls of 375/250 cols); DVE strided
     reduces finish the in-partition channel sum -> feats + proxy (fp32).
  3. DVE Gram products/reduce + PE ones-matmul broadcast + small-op chain
     -> per-(task,shot) cosine sims in every partition.
  4. Weighted shot sum with NO tensor-engine work: per task 2 ACT
     scale-copies, 2 DVE fused scalar-FMAs (in place), 1 GPSIMD FMA and
     1 GPSIMD add.
  Loads are issued on the SP DMA queue, stores on the ACT queue so store
  semaphore waits never block input prefetch (12-task double buffering).
"""

import numpy as np

import concourse.bacc as bacc
import concourse.mybir as mybir
import concourse.tile as tile
from concourse.bass_utils import run_bass_kernel_spmd

F32 = mybir.dt.float32
F16 = mybir.dt.float16
ADD = mybir.AluOpType.add
MULT = mybir.AluOpType.mult
COPY = mybir.ActivationFunctionType.Copy
X_AX = mybir.AxisListType.X

P = 128          # SBUF partitions
N = 5            # shots
C = 640          # channels
HW = 100         # 10*10 spatial
CF = C // P      # 5 channels per partition
FREE = CF * HW   # 500 floats per partition per (b, n)
OS = 25          # pooled spatial size (5*5)
SF = CF * OS     # 125: spatially-pooled cols per (b, n)
EPS = 1e-8
NCORES = 8
B = 256
BC = B // NCORES  # 32 tasks per core


def packs_np() -> np.ndarray:
    """(128, 512) fp16: 4 packing matrices.

    B4t routes channel-partition p of task t to oc row t*32 + p//4."""
    cs = np.zeros((P, 4 * P), np.float16)
    for t in range(4):
        for p in range(P):
            cs[p, t * 128 + t * 32 + p // 4] = 1.0
    return cs


def ones_np() -> np.ndarray:
    """(128, 512) fp32: 4 ones-blocks (cross-partition reduce+broadcast)."""
    cs = np.zeros((P, 4 * P), np.float32)
    for t in range(4):
        cs[32 * t:32 * (t + 1), t * 128:(t + 1) * 128] = 1.0
    return cs


def build(bc: int = BC, reps: int = 1):
    """Build + compile the per-core Bass module for a bc-task shard."""
    assert bc % 4 == 0
    nc = bacc.Bacc("TRN2", target_bir_lowering=False, debug=False,
                   num_devices=NCORES)
    x_in = nc.dram_tensor("x", (bc, N, C, HW), F32, kind="ExternalInput")
    pk_in = nc.dram_tensor("packs", (P, 4 * P), F16, kind="ExternalInput")
    on_in = nc.dram_tensor("ones", (P, 4 * P), F32, kind="ExternalInput")
    out_d = nc.dram_tensor("out", (bc, C, HW), F32, kind="ExternalOutput")

    xv = x_in[:].rearrange("b n (p cf) hw -> b p n (cf hw)", p=P, cf=CF)
    ov = out_d[:].rearrange("b (p cf) hw -> b p (cf hw)", p=P, cf=CF)

    with tile.TileContext(nc) as tc:
        with (
            tc.tile_pool(name="cpool", bufs=1) as cpool,
            tc.tile_pool(name="xpool", bufs=10) as xpool,
            tc.tile_pool(name="s1pool", bufs=3) as s1pool,
            tc.tile_pool(name="s2pool", bufs=6) as s2pool,
            tc.tile_pool(name="fpool", bufs=2) as fpool,
            tc.tile_pool(name="qpool", bufs=2) as qpool,
            tc.tile_pool(name="smpool", bufs=2) as smpool,
            tc.tile_pool(name="wpool", bufs=6) as wpool,
            tc.tile_pool(name="opool", bufs=6) as opool,
            tc.tile_pool(name="pkpool", bufs=2, space="PSUM") as pkpool,
            tc.tile_pool(name="rdpool", bufs=2, space="PSUM") as rdpool,
        ):
            pk16 = cpool.tile([P, 4 * P], F16)
            on32 = cpool.tile([P, 4 * P], F32)
            nc.sync.dma_start(pk16[:], pk_in[:])
            nc.sync.dma_start(on32[:], on_in[:])
            lhs_pack = [pk16[:, t * P:(t + 1) * P] for t in range(4)]
            lhs_ones = [on32[:, t * P:(t + 1) * P] for t in range(4)]

            for gi in range(reps * (bc // 4)):
                g = gi % (bc // 4)
                xts, s2ts = [], []
                for t in range(4):
                    xt = xpool.tile([P, N, FREE], F32, tag="x")
                    nc.sync.dma_start(xt[:], xv[4 * g + t])
                    xts.append(xt)
                    # 2x2 spatial pooling -> fp16 (s2 at 2x DVE rate)
                    s1 = s1pool.tile([P, N * CF * 50], F16, tag="s1")
                    v = xt[:].rearrange("p n (ci h wo dw) -> p (n ci) h wo dw",
                                        ci=CF, h=10, wo=5, dw=2)
                    nc.vector.tensor_tensor(
                        out=s1[:].rearrange("p (a h wo) -> p a h wo",
                                            a=N * CF, wo=5),
                        in0=v[:, :, :, :, 0], in1=v[:, :, :, :, 1], op=ADD)
                    s2 = s2pool.tile([P, N * SF], F16, tag="s2")
                    v1 = s1[:].rearrange("p (a ho dh wo) -> p a ho dh wo",
                                         a=N * CF, ho=5, dh=2)
                    nc.vector.tensor_tensor(
                        out=s2[:].rearrange("p (a ho wo) -> p a ho wo",
                                            a=N * CF, wo=5),
                        in0=v1[:, :, :, 0, :], in1=v1[:, :, :, 1, :], op=ADD)
                    s2ts.append(s2)

                # --- channel pooling: pack 4 tasks into PSUM, 2 banks ---
                pkA = pkpool.tile([P, 3 * SF], F32, tag="pkA")
                pkB = pkpool.tile([P, 2 * SF], F32, tag="pkB")
                for t in range(4):
                    nc.tensor.matmul(pkA[:], lhs_pack[t], s2ts[t][:, 0:3 * SF],
                                     start=(t == 0), stop=(t == 3))
                for t in range(4):
                    nc.tensor.matmul(pkB[:], lhs_pack[t], s2ts[t][:, 3 * SF:],
                                     start=(t == 0), stop=(t == 3))

                # FP: pooled feats [n0..n4] then proxy P at cols 125:150
                FP = fpool.tile([P, 6 * OS], F32, tag="FP")
                nc.vector.tensor_reduce(
                    out=FP[:, 0:3 * OS],
                    in_=pkA[:].rearrange("p (j ci s) -> p j s ci", j=3, ci=CF),
                    axis=X_AX, op=ADD)
                nc.vector.tensor_reduce(
                    out=FP[:, 3 * OS:5 * OS],
                    in_=pkB[:].rearrange("p (j ci s) -> p j s ci", j=2, ci=CF),
                    axis=X_AX, op=ADD)
                nc.vector.tensor_reduce(
                    out=FP[:, 5 * OS:6 * OS],
                    in_=FP[:, 0:5 * OS].rearrange("p (n s) -> p s n", n=N),
                    axis=X_AX, op=ADD)

                # --- Gram terms. QS cols: 0..4 <F_n,P>, 5 <P,P>, 6..10 <F_n,F_n>
                QP = qpool.tile([P, 11 * OS], F32, tag="QP")
                nc.vector.tensor_tensor(
                    out=QP[:, 0:6 * OS].rearrange("p (b s) -> p b s", b=6),
                    in0=FP[:].rearrange("p (b s) -> p b s", b=6),
                    in1=FP[:, 5 * OS:6 * OS].rearrange(
                        "p (b s) -> p b s", b=1).broadcast_to((P, 6, OS)),
                    op=MULT)
                nc.vector.tensor_tensor(
                    out=QP[:, 6 * OS:11 * OS], in0=FP[:, 0:5 * OS],
                    in1=FP[:, 0:5 * OS], op=MULT)
                QS = qpool.tile([P, 11], F32, tag="QS")
                nc.vector.tensor_reduce(
                    out=QS[:], in_=QP[:].rearrange("p (q s) -> p q s", q=11),
                    axis=X_AX, op=ADD)

                # --- cross-partition reduce + broadcast to all partitions ---
                rd = rdpool.tile([P, 44], F32, tag="rd")
                for t in range(4):
                    nc.tensor.matmul(rd[:, t * 11:(t + 1) * 11], lhs_ones[t],
                                     QS[:], start=True, stop=True)
                rsb = smpool.tile([P, 44], F32, tag="rsb")
                nc.scalar.activation(rsb[:], rd[:], COPY)
                rv = rsb[:].rearrange("p (t q) -> p t q", t=4)

                # --- cosine sims: sim = dot / max(sqrt(na2*nb2), eps) ---
                prod = smpool.tile([P, 20], F32, tag="prod")
                nc.vector.tensor_tensor(
                    out=prod[:].rearrange("p (t n) -> p t n", t=4),
                    in0=rv[:, :, 6:11],
                    in1=rv[:, :, 5:6].broadcast_to((P, 4, 5)), op=MULT)
                sq = smpool.tile([P, 20], F32, tag="sq")
                nc.scalar.activation(sq[:], prod[:],
                                     mybir.ActivationFunctionType.Sqrt)
                mx = smpool.tile([P, 20], F32, tag="mx")
                nc.vector.tensor_scalar_max(mx[:], sq[:], EPS)
                rs = smpool.tile([P, 20], F32, tag="rs")
                nc.vector.reciprocal(rs[:], mx[:])
                simt = smpool.tile([P, 20], F32, tag="simt")
                nc.vector.tensor_tensor(
                    out=simt[:].rearrange("p (t n) -> p t n", t=4),
                    in0=rv[:, :, 0:5],
                    in1=rs[:].rearrange("p (t n) -> p t n", t=4), op=MULT)

                # --- weighted sum of raw shots (ACT + DVE + GPSIMD) ---
                t0s, t2s, t4s = [], [], []
                for t in range(4):
                    sc = [simt[:, t * 5 + n:t * 5 + n + 1] for n in range(N)]
                    t0 = wpool.tile([P, FREE], F32, tag="t0")
                    nc.scalar.activation(t0[:], xts[t][:, 0, :], COPY,
                                         scale=sc[0])
                    t2 = wpool.tile([P, FREE], F32, tag="t2")
                    nc.scalar.activation(t2[:], xts[t][:, 2, :], COPY,
                                         scale=sc[2])
                    t4 = wpool.tile([P, FREE], F32, tag="t4")
                    nc.scalar.activation(t4[:], xts[t][:, 4, :], COPY,
                                         scale=sc[4])
                    nc.vector.scalar_tensor_tensor(
                        out=t0[:], in0=xts[t][:, 1, :], scalar=sc[1],
                        in1=t0[:], op0=MULT, op1=ADD)
                    nc.vector.scalar_tensor_tensor(
                        out=t2[:], in0=xts[t][:, 3, :], scalar=sc[3],
                        in1=t2[:], op0=MULT, op1=ADD)
                    t0s.append(t0)
                    t2s.append(t2)
                    t4s.append(t4)
                obs = []
                for t in range(4):
                    m = wpool.tile([P, FREE], F32, tag="m")
                    nc.gpsimd.tensor_tensor(out=m[:], in0=t0s[t][:],
                                            in1=t2s[t][:], op=ADD)
                    ob = opool.tile([P, FREE], F32, tag="ob")
                    nc.gpsimd.tensor_tensor(out=ob[:], in0=m[:],
                                            in1=t4s[t][:], op=ADD)
                    obs.append(ob)
                for t in range(4):
                    nc.scalar.dma_start(ov[4 * g + t], obs[t][:])

    nc.compile()
    return nc


_CACHE = {}


def _get_nc(bc: int = BC):
    if bc not in _CACHE:
        _CACHE[bc] = build(bc)
    return _CACHE[bc]


def in_map(xshard: np.ndarray) -> dict:
    return {"x": xshard, "packs": packs_np(), "ones": ones_np()}


def kernel(x: np.ndarray) -> np.ndarray:
    assert x.shape == (B, N, C, 10, 10) and x.dtype == np.float32
    nc = _get_nc(BC)
    shards = np.ascontiguousarray(x.reshape(NCORES, BC, N, C, HW))
    in_maps = [in_map(shards[i]) for i in range(NCORES)]
    res = run_bass_kernel_spmd(nc, in_maps, core_ids=list(range(NCORES)))
    out = np.concatenate([res.results[i]["out"] for i in range(NCORES)])
    return out.reshape(B, C, 10, 10).astype(np.float32)


# revision 11
# speedup vs baseline: 1.2325x; 1.2325x over previous
"""Trainium2 Bass kernel for nn_CosineProxy.

Reference computation (per task b):
    feats[n]  = blockmean_pool(x[b,n])            # (640,10,10) -> 800 dims
    proxy     = sum_n feats[n]                     # pooling is linear
    sim[n]    = <feats[n], proxy> / max(||feats[n]||*||proxy||, eps)
    out[b]    = sum_n sim[n] * x[b,n]

sim is scale-invariant, so block-SUM pooling is used instead of block-mean.
Sharding: pure data parallelism over B=256 tasks -> 32 tasks per core x 8 cores.

Per-core layout: x[b,n] (640*100 contiguous floats) lives in SBUF as
(128 partitions, 500 free) where partition p holds channels [5p,5p+5).

v2 pipeline per group of 4 tasks, tuned for the DMA roofline (~137us/core):
  1. DVE: 2x2 spatial pooling as two strided tensor_tensor adds per task,
     emitting fp16 (s2 runs at the 16-bit 2x DVE rate).
  2. PE: fp16 "packing" matmuls (lhsT = block-indicator) channel-pool the
     4 tasks into two PSUM banks (8 matmuls of 375/250 cols); DVE strided
     reduces finish the in-partition channel sum -> feats + proxy (fp32).
  3. DVE Gram products/reduce + PE ones-matmul broadcast + small-op chain
     -> per-(task,shot) cosine sims in every partition.
  4. Weighted shot sum with NO tensor-engine work: per task 2 ACT
     scale-copies, 2 DVE fused scalar-FMAs (in place), 1 GPSIMD FMA and
     1 GPSIMD add.
  Loads are issued on the SP DMA queue, stores on the ACT queue so store
  semaphore waits never block input prefetch (12-task double buffering).
"""

import numpy as np

import concourse.bacc as bacc
import concourse.mybir as mybir
import concourse.tile as tile
from concourse.bass_utils import run_bass_kernel_spmd

F32 = mybir.dt.float32
F16 = mybir.dt.float16
ADD = mybir.AluOpType.add
MULT = mybir.AluOpType.mult
COPY = mybir.ActivationFunctionType.Copy
X_AX = mybir.AxisListType.X

P = 128          # SBUF partitions
N = 5            # shots
C = 640          # channels
HW = 100         # 10*10 spatial
CF = C // P      # 5 channels per partition
FREE = CF * HW   # 500 floats per partition per (b, n)
OS = 25          # pooled spatial size (5*5)
SF = CF * OS     # 125: spatially-pooled cols per (b, n)
EPS = 1e-8
NCORES = 8
B = 256
BC = B // NCORES  # 32 tasks per core


def packs_np() -> np.ndarray:
    """(128, 640) fp16: 4 packing matrices + identity.

    B4t routes channel-partition p of task t to oc row t*32 + p//4."""
    cs = np.zeros((P, 5 * P), np.float16)
    for t in range(4):
        for p in range(P):
            cs[p, t * 128 + t * 32 + p // 4] = 1.0
    cs[np.arange(P), 4 * P + np.arange(P)] = 1.0
    return cs


def ones_np() -> np.ndarray:
    """(128, 512) fp32: 4 ones-blocks (cross-partition reduce+broadcast)."""
    cs = np.zeros((P, 4 * P), np.float32)
    for t in range(4):
        cs[32 * t:32 * (t + 1), t * 128:(t + 1) * 128] = 1.0
    return cs


def build(bc: int = BC, reps: int = 1):
    """Build + compile the per-core Bass module for a bc-task shard."""
    assert bc % 4 == 0
    nc = bacc.Bacc("TRN2", target_bir_lowering=False, debug=False,
                   num_devices=NCORES)
    x_in = nc.dram_tensor("x", (bc, N, C, HW), F32, kind="ExternalInput")
    pk_in = nc.dram_tensor("packs", (P, 5 * P), F16, kind="ExternalInput")
    on_in = nc.dram_tensor("ones", (P, 4 * P), F32, kind="ExternalInput")
    out_d = nc.dram_tensor("out", (bc, C, HW), F32, kind="ExternalOutput")

    xv = x_in[:].rearrange("b n (p cf) hw -> b p n (cf hw)", p=P, cf=CF)
    ov = out_d[:].rearrange("b (p cf) hw -> b p (cf hw)", p=P, cf=CF)

    with tile.TileContext(nc) as tc:
        with (
            tc.tile_pool(name="cpool", bufs=1) as cpool,
            tc.tile_pool(name="xpool", bufs=10) as xpool,
            tc.tile_pool(name="s1pool", bufs=3) as s1pool,
            tc.tile_pool(name="s2pool", bufs=6) as s2pool,
            tc.tile_pool(name="fpool", bufs=2) as fpool,
            tc.tile_pool(name="qpool", bufs=2) as qpool,
            tc.tile_pool(name="smpool", bufs=2) as smpool,
            tc.tile_pool(name="wpool", bufs=6) as wpool,
            tc.tile_pool(name="opool", bufs=6) as opool,
            tc.tile_pool(name="pkpool", bufs=1, space="PSUM") as pkpool,
            tc.tile_pool(name="rdpool", bufs=2, space="PSUM") as rdpool,
            tc.tile_pool(name="accpool", bufs=4, space="PSUM") as accpool,
        ):
            pk16 = cpool.tile([P, 5 * P], F16)
            on32 = cpool.tile([P, 4 * P], F32)
            nc.sync.dma_start(pk16[:], pk_in[:])
            nc.sync.dma_start(on32[:], on_in[:])
            lhs_pack = [pk16[:, t * P:(t + 1) * P] for t in range(4)]
            lhs_eye = pk16[:, 4 * P:5 * P]
            lhs_ones = [on32[:, t * P:(t + 1) * P] for t in range(4)]

            for gi in range(reps * (bc // 4)):
                g = gi % (bc // 4)
                xts, s2ts = [], []
                for t in range(4):
                    xt = xpool.tile([P, N, FREE], F32, tag="x")
                    nc.sync.dma_start(xt[:], xv[4 * g + t])
                    xts.append(xt)
                    # 2x2 spatial pooling -> fp16 (s2 at 2x DVE rate)
                    s1 = s1pool.tile([P, N * CF * 50], F16, tag="s1")
                    v = xt[:].rearrange("p n (ci h wo dw) -> p (n ci) h wo dw",
                                        ci=CF, h=10, wo=5, dw=2)
                    nc.vector.tensor_tensor(
                        out=s1[:].rearrange("p (a h wo) -> p a h wo",
                                            a=N * CF, wo=5),
                        in0=v[:, :, :, :, 0], in1=v[:, :, :, :, 1], op=ADD)
                    s2 = s2pool.tile([P, N * SF], F16, tag="s2")
                    v1 = s1[:].rearrange("p (a ho dh wo) -> p a ho dh wo",
                                         a=N * CF, ho=5, dh=2)
                    nc.vector.tensor_tensor(
                        out=s2[:].rearrange("p (a ho wo) -> p a ho wo",
                                            a=N * CF, wo=5),
                        in0=v1[:, :, :, 0, :], in1=v1[:, :, :, 1, :], op=ADD)
                    s2ts.append(s2)

                # --- channel pooling: pack 4 tasks into PSUM, 2 banks ---
                pkA = pkpool.tile([P, 3 * SF], F32, tag="pkA")
                pkB = pkpool.tile([P, 2 * SF], F32, tag="pkB")
                for t in range(4):
                    nc.tensor.matmul(pkA[:], lhs_pack[t], s2ts[t][:, 0:3 * SF],
                                     start=(t == 0), stop=(t == 3))
                for t in range(4):
                    nc.tensor.matmul(pkB[:], lhs_pack[t], s2ts[t][:, 3 * SF:],
                                     start=(t == 0), stop=(t == 3))

                # FP: pooled feats [n0..n4] then proxy P at cols 125:150
                FP = fpool.tile([P, 6 * OS], F32, tag="FP")
                nc.vector.tensor_reduce(
                    out=FP[:, 0:3 * OS],
                    in_=pkA[:].rearrange("p (j ci s) -> p j s ci", j=3, ci=CF),
                    axis=X_AX, op=ADD)
                nc.vector.tensor_reduce(
                    out=FP[:, 3 * OS:5 * OS],
                    in_=pkB[:].rearrange("p (j ci s) -> p j s ci", j=2, ci=CF),
                    axis=X_AX, op=ADD)
                nc.vector.tensor_reduce(
                    out=FP[:, 5 * OS:6 * OS],
                    in_=FP[:, 0:5 * OS].rearrange("p (n s) -> p s n", n=N),
                    axis=X_AX, op=ADD)

                # --- Gram terms. QS cols: 0..4 <F_n,P>, 5 <P,P>, 6..10 <F_n,F_n>
                QP = qpool.tile([P, 11 * OS], F32, tag="QP")
                nc.vector.tensor_tensor(
                    out=QP[:, 0:6 * OS].rearrange("p (b s) -> p b s", b=6),
                    in0=FP[:].rearrange("p (b s) -> p b s", b=6),
                    in1=FP[:, 5 * OS:6 * OS].rearrange(
                        "p (b s) -> p b s", b=1).broadcast_to((P, 6, OS)),
                    op=MULT)
                nc.vector.tensor_tensor(
                    out=QP[:, 6 * OS:11 * OS], in0=FP[:, 0:5 * OS],
                    in1=FP[:, 0:5 * OS], op=MULT)
                QS = qpool.tile([P, 11], F32, tag="QS")
                nc.vector.tensor_reduce(
                    out=QS[:], in_=QP[:].rearrange("p (q s) -> p q s", q=11),
                    axis=X_AX, op=ADD)

                # --- cross-partition reduce + broadcast to all partitions ---
                rd = rdpool.tile([P, 44], F32, tag="rd")
                for t in range(4):
                    nc.tensor.matmul(rd[:, t * 11:(t + 1) * 11], lhs_ones[t],
                                     QS[:], start=True, stop=True)
                rsb = smpool.tile([P, 44], F32, tag="rsb")
                nc.scalar.activation(rsb[:], rd[:], COPY)
                rv = rsb[:].rearrange("p (t q) -> p t q", t=4)

                # --- cosine sims: sim = dot / max(sqrt(na2*nb2), eps) ---
                prod = smpool.tile([P, 20], F32, tag="prod")
                nc.vector.tensor_tensor(
                    out=prod[:].rearrange("p (t n) -> p t n", t=4),
                    in0=rv[:, :, 6:11],
                    in1=rv[:, :, 5:6].broadcast_to((P, 4, 5)), op=MULT)
                sq = smpool.tile([P, 20], F32, tag="sq")
                nc.scalar.activation(sq[:], prod[:],
                                     mybir.ActivationFunctionType.Sqrt)
                mx = smpool.tile([P, 20], F32, tag="mx")
                nc.vector.tensor_scalar_max(mx[:], sq[:], EPS)
                rs = smpool.tile([P, 20], F32, tag="rs")
                nc.vector.reciprocal(rs[:], mx[:])
                simt = smpool.tile([P, 20], F32, tag="simt")
                nc.vector.tensor_tensor(
                    out=simt[:].rearrange("p (t n) -> p t n", t=4),
                    in0=rv[:, :, 0:5],
                    in1=rs[:].rearrange("p (t n) -> p t n", t=4), op=MULT)

                # --- weighted sum of raw shots ---
                # shots 0-3: ACT scale-copy to fp16, PE identity-matmul
                # accumulate in PSUM; shot 4: DVE fused FMA reading PSUM.
                obs = []
                for t in range(4):
                    sc = [simt[:, t * 5 + n:t * 5 + n + 1] for n in range(N)]
                    acc = accpool.tile([P, FREE], F32, tag="acc")
                    for n in range(4):
                        cn = wpool.tile([P, FREE], F16, tag=f"c{n}")
                        nc.scalar.activation(cn[:], xts[t][:, n, :], COPY,
                                             scale=sc[n])
                        nc.tensor.matmul(acc[:], lhs_eye, cn[:],
                                         start=(n == 0), stop=(n == 3))
                    ob = opool.tile([P, FREE], F32, tag="ob")
                    nc.vector.scalar_tensor_tensor(
                        out=ob[:], in0=xts[t][:, 4, :], scalar=sc[4],
                        in1=acc[:], op0=MULT, op1=ADD)
                    obs.append(ob)
                for t in range(4):
                    nc.scalar.dma_start(ov[4 * g + t], obs[t][:])

    nc.compile()
    return nc


_CACHE = {}


def _get_nc(bc: int = BC):
    if bc not in _CACHE:
        _CACHE[bc] = build(bc)
    return _CACHE[bc]


def in_map(xshard: np.ndarray) -> dict:
    return {"x": xshard, "packs": packs_np(), "ones": ones_np()}


def kernel(x: np.ndarray) -> np.ndarray:
    assert x.shape == (B, N, C, 10, 10) and x.dtype == np.float32
    nc = _get_nc(BC)
    shards = np.ascontiguousarray(x.reshape(NCORES, BC, N, C, HW))
    in_maps = [in_map(shards[i]) for i in range(NCORES)]
    res = run_bass_kernel_spmd(nc, in_maps, core_ids=list(range(NCORES)))
    out = np.concatenate([res.results[i]["out"] for i in range(NCORES)])
    return out.reshape(B, C, 10, 10).astype(np.float32)
